# revision 40
# baseline (speedup 1.0000x reference)
"""Trainium2 Bass kernel for nn_Attention_85796266705382.

Reference computation (per batch element, b=8, HEAD=8, n=32*32=1024, c=dim=512):
    qkv = x @ w_qkv                      # (n, 1536), per-head interleaved [q|k|v] x 64
    q,k,v per head (n, 64)
    attn = softmax(q @ k.T * 8**-0.5)    # scale uses FULL batch size (reference quirk)
    out  = attn @ v                      # (n, 64) per head -> (n, 512)
    y    = out @ w_out + b_out           # (n, 512)

Sharding: pure data-parallel over batch — one batch element per NeuronCore (8 cores).

Per-core device strategy (v2 — pair-lagged pipeline, PE row-tile concurrency):
  * Inputs x / w_qk / w_v in bf16 (halves the input DMA; max rel err ~7e-3
    vs the 2e-2 budget). First-needed column chunks are loaded first so the
    qk projection starts ~2.5us in.
  * qk-pass: qkT [1024, n] = w_qk.T @ xT with host-permuted w_qk columns
    [q_h0 q_h1 k_h0 k_h1 | q_h2 ...] so each 128-row tile is a head-PAIR band
    (rows 0-63 head even / 64-127 head odd).
  * scores (transposed) sT_h [j, i] = kT_h.T @ qT_h per 128-row j-tile. The
    two heads of a pair are emitted back-to-back as (64,128) row-tiled
    matmuls on PE tiles T0/T8 — they execute CONCURRENTLY on the split
    128x128 array (2x effective score throughput).
  * exp on ScalarE reads PSUM, folds the 8**-0.5 scale, writes bf16 to SBUF.
  * AV for pair p-1 interleaves with scores for pair p (one-pair lag) so the
    exp latency never sits on the PE critical path.
  * v-pass: v natural [n, 512] = xT.T @ w_v stored with 65-column per-head
    pitch, column 64 = ones (attention-weight row sums fall out of the AV
    matmul for free).
  * normalization without DRAM bounce: DVE reciprocal of the PSUM denom row,
    then a matmul against a one-hot stationary broadcasts it across 128
    partitions; DVE multiplies the AV rows.
  * final projection split: pairs 0-2 accumulate into PSUM while pair 3's AV
    still runs; after pair-3 norm only 8 matmuls + bias adds + stores remain.
"""

import numpy as np


def _ensure_paths():
    import sys

    try:
        import concourse.bass  # noqa: F401

        return
    except ImportError:
        pass
    for p in ("/opt/trn_rl_repo", "/root/.axon_site/_ro/trn_rl_repo"):
        if p not in sys.path:
            sys.path.append(p)
    import concourse.bass  # noqa: F401


HEAD = 8
B = 8
N = 1024  # tokens per batch element (32*32)
C = 512  # channels
DIM = 512
DH = 64
SCALE = float(B) ** -0.5  # reference scales by batch size, reproduced faithfully
N_CORES = 8

_CACHE = {}


def _split_excess_waits(nc, mybir, bass_rust):
    """walrus in this container accepts 1 sync wait per instruction (2 for
    EventSemaphore); Tile sometimes attaches more. Hoist the excess onto fresh
    same-engine NoOps inserted just before the over-capacity instruction —
    same-engine program order preserves the synchronization semantics."""
    n_split = 0
    for fn in nc.m.functions:
        for bb in fn.blocks:
            insts = bb.instructions
            i = 0
            while i < len(insts):
                inst = insts[i]
                si = inst.sync_info
                cap = 2 if isinstance(inst, mybir.InstEventSemaphore) else 1
                if si is not None and len(si.on_wait) > cap:
                    extra = list(si.on_wait[cap:])
                    del si.on_wait[cap:]
                    new_insts = []
                    # EventSemaphore NOPs hold 2 waits each -> fewer queue bubbles
                    for k in range(0, len(extra), 2):
                        pair = extra[k : k + 2]
                        nop = mybir.InstEventSemaphore(
                            name=f"{inst.name}_ws{k}", ins=[], outs=[]
                        )
                        nop.engine = inst.engine
                        nop.sync_info = bass_rust.SyncInfo(on_wait=pair, on_update=[])
                        new_insts.append(nop)
                        n_split += 1
                    insts[i:i] = new_insts
                    i += len(new_insts)
                i += 1
    return n_split


def _build():
    if "nc" in _CACHE:
        return _CACHE["nc"]
    _ensure_paths()
    import bass_rust
    import concourse.bass as bass
    import concourse.mybir as mybir
    import concourse.tile as tile

    f32 = mybir.dt.float32
    f32r = mybir.dt.float32r
    bf16 = mybir.dt.bfloat16
    u16 = mybir.dt.uint16
    u32 = mybir.dt.uint32
    Exp = mybir.ActivationFunctionType.Exp

    nc = bass.Bass(trn_type="TRN2", target_bir_lowering=False, debug=False)

    xT_d = nc.dram_tensor("xT", [C, N], bf16, kind="ExternalInput").ap()
    wqk_d = nc.dram_tensor("w_qk", [C, 2 * DIM], bf16, kind="ExternalInput").ap()
    wv_d = nc.dram_tensor("w_v", [C, DIM], bf16, kind="ExternalInput").ap()
    wo_d = nc.dram_tensor("w_out", [DIM, DIM], f32r, kind="ExternalInput").ap()
    b_d = nc.dram_tensor("b_out", [DIM], f32, kind="ExternalInput").ap()
    out_d = nc.dram_tensor("out", [N, DIM], f32, kind="ExternalOutput").ap()
    rd_d = nc.dram_tensor("rd_scratch", [2, N], f32r).ap()

    with tile.TileContext(nc) as tc:
        with (
            tc.tile_pool(name="wp", bufs=1) as wp,
            tc.tile_pool(name="xp", bufs=1) as xp,
            tc.tile_pool(name="qkp", bufs=1) as qkp,
            tc.tile_pool(name="vpp", bufs=1) as vpp,
            tc.tile_pool(name="ptp", bufs=28) as ptp,
            tc.tile_pool(name="otp", bufs=1) as otp,
            tc.tile_pool(name="rdp", bufs=1) as rdp,
            tc.tile_pool(name="bcp", bufs=1) as bcp,
            tc.tile_pool(name="ysp", bufs=1) as ysp,
            tc.tile_pool(name="obp", bufs=3) as obp,
            tc.tile_pool(name="psS", bufs=2, space="PSUM") as psS,
            tc.tile_pool(name="psB", bufs=4, space="PSUM") as psB,
        ):
            # ---- input loads: first-needed chunks first; HWDGE drains the
            # queue FIFO so later loads never delay the first matmuls ----
            xT, wqk = [], []
            for ct in range(4):
                xT.append(xp.tile([128, N], bf16, tag=f"xT{ct}", name=f"xT{ct}"))
                wqk.append(
                    wp.tile([128, 2 * DIM], bf16, tag=f"wqk{ct}", name=f"wqk{ct}")
                )
            for ct in range(4):
                nc.sync.dma_start(
                    out=wqk[ct][:, 0:256], in_=wqk_d[ct * 128 : (ct + 1) * 128, 0:256]
                )
                nc.sync.dma_start(
                    out=xT[ct][:, 0:512], in_=xT_d[ct * 128 : (ct + 1) * 128, 0:512]
                )
            for ct in range(4):
                nc.sync.dma_start(
                    out=xT[ct][:, 512:1024],
                    in_=xT_d[ct * 128 : (ct + 1) * 128, 512:1024],
                )
                nc.sync.dma_start(
                    out=wqk[ct][:, 256:1024],
                    in_=wqk_d[ct * 128 : (ct + 1) * 128, 256:1024],
                )
            wv = []
            for ct in range(4):
                t = wp.tile([128, DIM], bf16, tag=f"wv{ct}", name=f"wv{ct}")
                nc.sync.dma_start(out=t[:], in_=wv_d[ct * 128 : (ct + 1) * 128, :])
                wv.append(t)

            def xTs(ct, a, b):
                return xT[ct][:, a:b]

            def wqks(ct, a, b):
                return wqk[ct][:, a:b]

            # one-hot stationary (row 0 = 1.0) for the pair-3 bc matmul
            ones_mm = wp.tile([128, 128], f32r, tag="ones_mm", name="ones_mm")
            nc.vector.memset(ones_mm[:].bitcast(u32), 0)
            nc.vector.memset(ones_mm[0:1, :].bitcast(u32), 1065353216)

            # den-transpose scratch per which: [32, 32blk, 32] viewing the
            # 32x32 stream-transpose blocks; reciprocal then runs on 16
            # strided columns instead of 512-wide single-partition rows
            trr = [
                rdp.tile([32, 32, 32], f32, tag=f"trr{w}", name=f"trr{w}")
                for w in range(2)
            ]
            trq = [
                rdp.tile([32, 32, 32], f32, tag=f"trq{w}", name=f"trq{w}")
                for w in range(2)
            ]
            rdT = [
                rdp.tile([32, 32, 32], f32, tag=f"rdT{w}", name=f"rdT{w}")
                for w in range(2)
            ]
            rdrow = []
            for w in range(2):
                t = rdp.tile([128, N], f32r, tag=f"rdr{w}", name=f"rdr{w}")
                nc.vector.memset(t[:].bitcast(u32), 0)
                rdrow.append(t)

            # zero-padded K-tiles: kp[par][w] holds head-w's K rows in the
            # same 64-row band as the Q tile layout, other 64 rows ZERO, so
            # score matmuls contract over the full 128 partitions (the other
            # head's q rows hit zero weights -> exact). Every matmul in the
            # kernel is then tile_size (128,128): no PE mode switches, and
            # LDWEIGHTS always hides in the background weight buffer.
            kp = {}
            for par in range(2):
                for w in range(2):
                    t = qkp.tile([128, N], f32r, tag=f"kp{par}{w}", name=f"kp{par}{w}")
                    if w == 0:
                        nc.vector.memset(t[64:128, :].bitcast(u32), 0)
                    else:
                        nc.vector.memset(t[0:64, :].bitcast(u32), 0)
                    kp[(par, w)] = t

            def qk_tile(dt_i):
                """one d-tile of the transposed qk projection -> SBUF f32r.
                Odd d-tiles (K) are written as the zero-padded kp pair."""
                ps = psS.tile([128, N], f32, tag="S", name=f"qkps{dt_i}")
                for ct in range(4):
                    for ch in range(2):
                        nc.tensor.matmul(
                            ps[:, ch * 512 : (ch + 1) * 512],
                            wqks(ct, dt_i * 128, (dt_i + 1) * 128),
                            xTs(ct, ch * 512, (ch + 1) * 512),
                            start=(ct == 0),
                            stop=(ct == 3),
                        )
                if dt_i % 2 == 0:
                    t = qkp.tile([128, N], f32r, tag=f"qk{dt_i}", name=f"qk{dt_i}")
                    nc.vector.tensor_copy(t[:], ps[:])
                    return t
                par = (dt_i // 2) % 2
                nc.vector.tensor_copy(kp[(par, 0)][0:64, :], ps[0:64, :])
                nc.vector.tensor_copy(kp[(par, 1)][64:128, :], ps[64:128, :])
                return par

            def scores_jt(p, jt, QT, par):
                """score matmuls (zero-padded K=128) + exp for one (pair, jt)"""
                sps = [
                    psS.tile([128, N], f32, tag="S", name=f"s_{p}_{jt}_{w}")
                    for w in range(2)
                ]
                for w in range(2):
                    for ch in range(2):
                        nc.tensor.matmul(
                            sps[w][:, ch * 512 : (ch + 1) * 512],
                            kp[(par, w)][:, jt * 128 : (jt + 1) * 128],
                            QT[:, ch * 512 : (ch + 1) * 512],
                            start=True,
                            stop=True,
                        )
                pts = []
                for w in range(2):
                    pt = ptp.tile([128, N], bf16, tag="pt", name=f"pt{p}_{jt}_{w}")
                    nc.scalar.activation(pt[:], sps[w][:], Exp, scale=SCALE)
                    pts.append(pt)
                return pts

            def v_jt(jt):
                """one j-tile of the v projection, 65-pitch + ones column"""
                vt = vpp.tile([128, HEAD, DH + 1], bf16, tag=f"v{jt}", name=f"v{jt}")
                nc.vector.memset(vt[:, :, DH : DH + 1].bitcast(u16), 16256)
                ps = psB.tile([128, 512], f32, tag="B", name=f"vps{jt}")
                for ct in range(4):
                    nc.tensor.matmul(
                        ps[:],
                        xTs(ct, jt * 128, (jt + 1) * 128),
                        wv[ct][:],
                        start=(ct == 0),
                        stop=(ct == 3),
                    )
                nc.vector.tensor_copy(
                    vt[:, :, 0:DH],
                    ps[:].rearrange("p (h e) -> p h e", h=HEAD),
                )
                return vt

            def make_av(p):
                # full 128-partition PSUM tiles: rows 0-64 = AV out + denom,
                # rows 65-95 slice feeds the 32-aligned den transpose
                return [
                    [
                        psB.tile([128, 512], f32, tag="B", name=f"av{p}_{w}_{c}")
                        for c in range(2)
                    ]
                    for w in range(2)
                ]

            def av_jt(p, jt, av, pts, which=(0, 1)):
                for w in which:
                    for ch in range(2):
                        nc.tensor.matmul(
                            av[w][ch][0 : DH + 1, :],
                            v_sb[jt][:, 2 * p + w, :],
                            pts[w][:, ch * 512 : (ch + 1) * 512],
                            start=(jt == 0),
                            stop=(jt == 7),
                        )

            def norm_den(p, av, w):
                """DVE chain: den row (PSUM partition 64) -> 1/den row.
                Stream-transpose of rows 64:96 drops den at column 0 of each
                32x32 block; reciprocal (16 strided cols) writes trq; the
                transpose back yields a [1, 512] row of 1/den at partition 0,
                rounded to f32r in rdrow."""
                for ch in range(2):
                    blk = slice(ch * 16, (ch + 1) * 16)
                    nc.vector.transpose(
                        trr[w][:, blk, :].rearrange("p a b -> p (a b)"),
                        av[w][ch][64:96, :],
                    )
                    with nc.allow_low_precision(reason="f32r == fp32 bits"):
                        nc.vector.reciprocal(trq[w][:, blk, 0:1], trr[w][:, blk, 0:1])
                    nc.vector.transpose(
                        rdT[w][:, blk, :].rearrange("p a b -> p (a b)"),
                        trq[w][:, blk, :].rearrange("p a b -> p (a b)"),
                    )
                nc.vector.tensor_copy(
                    rdrow[w][0:1, :], rdT[w][0:1, :, :].rearrange("p a b -> p (a b)")
                )

            def norm_apply(p, av, w, ot):
                """partition-broadcast of the 1/den row, then the
                normalizing multiplies. Pairs 0-2: bounce through DRAM and
                re-read with a stride-0 partition AP (no PE/ACT cost, the
                DMA latency hides under the next round). Pair 3: broadcast
                via one-hot matmul + ScalarE copy (shortest tail latency)."""
                bc_sb = bcp.tile([64, N], f32r, tag=f"bc{w}", name=f"bcs{p}_{w}")
                bc_ps = psS.tile([128, N], f32, tag="S", name=f"bc{p}_{w}")
                for ch in range(2):
                    nc.tensor.matmul(
                        bc_ps[:, ch * 512 : (ch + 1) * 512],
                        ones_mm[:],
                        rdrow[w][:, ch * 512 : (ch + 1) * 512],
                        start=True,
                        stop=True,
                    )
                nc.scalar.activation(
                    bc_sb[:], bc_ps[0:64, :], mybir.ActivationFunctionType.Copy
                )
                for ch in range(2):
                    nc.vector.tensor_mul(
                        ot[w * 64 : (w + 1) * 64, ch * 512 : (ch + 1) * 512],
                        av[w][ch][0:DH, :],
                        bc_sb[:, ch * 512 : (ch + 1) * 512],
                    )

            def norm_pair(p, av, ot=None):
                if ot is None:
                    ot = otp.tile([128, N], f32r, tag=f"ot{p}", name=f"ot{p}")
                for w in range(2):
                    norm_den(p, av, w)
                    norm_apply(p, av, w, ot)
                return ot

            # ================= round 0: pair-0 projection + scores + v ======
            qk = {}
            qk[0] = qk_tile(0)
            par0 = qk_tile(1)

            v_sb = [None] * 8
            pts_prev = [None] * 8
            for jt in range(8):
                pts_prev[jt] = scores_jt(0, jt, qk[0], par0)
                v_sb[jt] = v_jt(jt)

            qk[2] = qk_tile(2)
            qk[3] = qk_tile(3)  # returns parity for pair 1

            ot_tiles = [None] * 4
            av_prev = None

            # ================= rounds 1-3: scores p | AV p-1 ================
            for p in range(1, 4):
                QT, par = qk[2 * p], qk[2 * p + 1]
                av = make_av(p - 1)
                pts_cur = [None] * 8
                for jt in range(8):
                    pts_cur[jt] = scores_jt(p, jt, QT, par)
                    av_jt(p - 1, jt, av, pts_prev[jt])
                # projection / loads for the NEXT pair come before the norm so
                # the in-order DVE queue serves the qk copies first
                if p < 3:
                    qk[2 * p + 2] = qk_tile(2 * p + 2)
                    qk[2 * p + 3] = qk_tile(2 * p + 3)
                else:
                    # late weight loads (needed only by the final projection)
                    wo = []
                    for p4 in range(4):
                        t = wp.tile([128, DIM], f32r, tag=f"wo{p4}", name=f"wo{p4}")
                        nc.sync.dma_start(
                            out=t[:], in_=wo_d[p4 * 128 : (p4 + 1) * 128, :]
                        )
                        wo.append(t)
                    bb_t = wp.tile([128, DIM], f32, tag="bb", name="bb")
                    b_src = bass.AP(
                        tensor=b_d.tensor,
                        offset=b_d.offset,
                        ap=[[0, 128]] + list(b_d.ap),
                    )
                    nc.sync.dma_start(out=bb_t[:], in_=b_src)
                ot_tiles[p - 1] = norm_pair(p - 1, av)
                pts_prev = pts_cur

            # ======== round 4: AV p3 (w-split) | norm3 | partials p0-2 ======
            av3 = make_av(3)
            for it in range(8):
                av_jt(3, it, av3, pts_prev[it], which=(0,))
            ot3 = otp.tile([128, N], f32r, tag="ot3", name="ot3")
            norm_den(3, av3, 0)
            for it in range(8):
                av_jt(3, it, av3, pts_prev[it], which=(1,))
            norm_den(3, av3, 1)
            norm_apply(3, av3, 0, ot3)

            def partial_it(it):
                fps = psS.tile([128, 512], f32, tag="S", name=f"fp{it}")
                for p4 in range(3):
                    nc.tensor.matmul(
                        fps[:],
                        ot_tiles[p4][:, it * 128 : (it + 1) * 128],
                        wo[p4][:],
                        start=(p4 == 0),
                        stop=(p4 == 2),
                    )
                ys = ysp.tile([128, DIM], f32, tag=f"ys{it}", name=f"ys{it}")
                nc.vector.tensor_add(ys[:], fps[:], bb_t[:])
                return ys

            ysub = [None] * 8
            for it in range(4):
                ysub[it] = partial_it(it)
            norm_apply(3, av3, 1, ot3)
            ot_tiles[3] = ot3

            # tiles 4-7: ot3 is ready — fuse all four pairs in one pass
            for it in range(4, 8):
                fps = psS.tile([128, 512], f32, tag="S", name=f"ff{it}")
                for p4 in range(4):
                    nc.tensor.matmul(
                        fps[:],
                        ot_tiles[p4][:, it * 128 : (it + 1) * 128],
                        wo[p4][:],
                        start=(p4 == 0),
                        stop=(p4 == 3),
                    )
                os_t = obp.tile([128, DIM], f32, tag="os", name=f"os{it}")
                nc.vector.tensor_add(os_t[:], fps[:], bb_t[:])
                nc.sync.dma_start(out=out_d[it * 128 : (it + 1) * 128, :], in_=os_t[:])

            # ================= tail: pair-3 for tiles 0-3 ===================
            for it in range(4):
                fps = psS.tile([128, 512], f32, tag="S", name=f"fq{it}")
                nc.tensor.matmul(
                    fps[:],
                    ot_tiles[3][:, it * 128 : (it + 1) * 128],
                    wo[3][:],
                    start=True,
                    stop=True,
                )
                os_t = obp.tile([128, DIM], f32, tag="os", name=f"os{it}")
                nc.vector.tensor_add(os_t[:], fps[:], ysub[it][:])
                nc.sync.dma_start(out=out_d[it * 128 : (it + 1) * 128, :], in_=os_t[:])

    _split_excess_waits(nc, mybir, bass_rust)
    _CACHE["nc"] = nc
    return nc


def _prep_inputs(inputs):
    import ml_dtypes

    bfnp = ml_dtypes.bfloat16
    x = np.ascontiguousarray(inputs["x"], dtype=np.float32)
    w_qkv = np.ascontiguousarray(inputs["w_qkv"], dtype=np.float32)
    w_out = np.ascontiguousarray(inputs["w_out"], dtype=np.float32)
    b_out = np.ascontiguousarray(inputs["b_out"], dtype=np.float32)

    # per-head slices of the fused qkv weight
    wq = [w_qkv[:, h * 192 : h * 192 + 64] for h in range(HEAD)]
    wk = [w_qkv[:, h * 192 + 64 : h * 192 + 128] for h in range(HEAD)]
    wv = [w_qkv[:, h * 192 + 128 : h * 192 + 192] for h in range(HEAD)]
    # pair-banded column order: [q0 q1 k0 k1 | q2 q3 k2 k3 | ...]
    blocks = []
    for p in range(4):
        blocks += [wq[2 * p], wq[2 * p + 1], wk[2 * p], wk[2 * p + 1]]
    w_qk = np.ascontiguousarray(np.concatenate(blocks, axis=1)).astype(bfnp)
    w_v = np.ascontiguousarray(np.concatenate(wv, axis=1)).astype(bfnp)

    in_maps = []
    for i in range(N_CORES):
        xT = np.ascontiguousarray(x[i].reshape(N, C).T).astype(bfnp)
        in_maps.append(
            {"xT": xT, "w_qk": w_qk, "w_v": w_v, "w_out": w_out, "b_out": b_out}
        )
    return in_maps


def _run(inputs, trace=False):
    _ensure_paths()
    import os

    if trace:
        os.environ.pop("BASS_NEVER_TRACE", None)
    else:
        # keep run_bass_kernel_spmd off the NTFF-profile path (its hook import
        # is environment-dependent); correctness runs never need tracing
        os.environ["BASS_NEVER_TRACE"] = "1"
    from concourse import bass_utils

    nc = _build()
    in_maps = _prep_inputs(inputs)
    res = bass_utils.run_bass_kernel_spmd(
        nc, in_maps, core_ids=list(range(N_CORES)), trace=trace
    )
    out = np.stack(
        [res.results[i]["out"].reshape(32, 32, DIM) for i in range(N_CORES)]
    ).astype(np.float32)
    return out, res


def kernel(**inputs):
    out, _ = _run(inputs, trace=False)
    return out
